# revision 7
# baseline (speedup 1.0000x reference)
"""Trainium2 Bass kernel for nn_Attention_4363686773373.

Sigmoid attention with magnitude-preserving (weight-normalized) projections.

Sharding: data-parallel over (batch, T-half) -> 8 shards on 8 NeuronCores.
Each core computes q for its 1024 tokens and k,v for the full 2048 tokens of
its batch (k/v recomputed on both cores of a batch; no collectives).

Per-core dataflow (all heavy matmuls in bf16 with fp32 PSUM accumulation):
  W: row-normalize qkv_w / out_w on device, transpose via DMA xbar -> wnT, ownT
  X: cast x to bf16 (gpsimd cast-DMA), transpose -> xkvT; per-token ||x|| -> mag
  A: qkv projection (natural [t,e] layout), q/k cosine-normalize along head_dim
     (free-dim reduce), transpose qn/kn -> [head_dim, t] layout
  B: per head: scoresT = knT^T qnT (K=64 row-packed pairs), sigmoid on ACT
     (PSUM->SBUF bf16), attnT @ v accumulated over key blocks -> avT
  C: normalize av per (token, head), scale by token magnitude, out-projection.
"""

import math
from contextlib import ExitStack

import numpy as np

import concourse.bass as bass
import concourse.tile as tile
from concourse import bacc, mybir
from concourse.bass_utils import run_bass_kernel_spmd


def _ensure_axon_hooks():
    """This image's antenv lacks axon_hooks; reconstruct it so trace=True
    (NTFF profiling) works instead of crashing on import."""
    try:
        import antenv.axon_hooks  # noqa: F401
        return
    except ImportError:
        pass
    import sys
    import types
    try:
        import antenv
    except ImportError:
        return
    mod = types.ModuleType("antenv.axon_hooks")
    _hook = [None]
    mod.set_axon_ntff_profile_hook = lambda h: _hook.__setitem__(0, h)
    mod.get_axon_ntff_profile_hook = lambda: _hook[0]
    sys.modules["antenv.axon_hooks"] = mod
    antenv.axon_hooks = mod
    try:
        from trn_agent_boot.trn_boot import _ntff_profile_via_ctypes
        mod.set_axon_ntff_profile_hook(
            _ntff_profile_via_ctypes('/opt/axon/libaxon_pjrt.so'))
    except Exception:
        pass


_ensure_axon_hooks()

# Problem shapes (hardcoded per harness contract)
B, T, D, H = 4, 2048, 768, 12
HD = D // H  # 64
EPS = 1e-4
SIGMOID_GAIN = 1.8402
N_CORES = 8

F32 = mybir.dt.float32
BF16 = mybir.dt.bfloat16
AF = mybir.ActivationFunctionType
ALU = mybir.AluOpType
AX = mybir.AxisListType


def _chunks(total, maxn=512):
    out = []
    c0 = 0
    while c0 < total:
        cn = min(maxn, total - c0)
        out.append((c0, cn))
        c0 += cn
    return out


def build_program(nc, tc, ctx, Tq, Tkv, Dm, Hn):
    """Emit the per-core program. xkv rows are pre-ordered so the first Tq
    tokens are this core's query tokens (attention is permutation-invariant
    over the key axis)."""
    keep = []  # keep tc.tile free-closures alive (GC would release the pools)

    def _tile(shape, dtype, name):
        t, free = tc.tile(shape, dtype, name=name)
        keep.append(free)
        return t, free

    tc._ant_keepalive = keep
    P = 128
    HDl = 64
    assert Dm % P == 0 and Tq % P == 0 and Tkv % P == 0
    DT = Dm // P          # d-tiles
    E3 = 3 * Dm
    PAIRS = Hn // 2       # head pairs; pair = 128 contiguous features
    assert PAIRS * P == Dm and Hn * HDl == Dm
    TBq = Tq // P
    TBkv = Tkv // P
    WE = E3 // P          # qkv_w row tiles
    # eps seen by the post-attention normalize, after folding out the
    # gain/sqrt(T) prefactor (we accumulate raw attn@v).
    eps_av = EPS * math.sqrt(Tkv) / SIGMOID_GAIN

    xkv = nc.dram_tensor("xkv", [Tkv, Dm], F32, kind="ExternalInput").ap()
    qkvw = nc.dram_tensor("qkvw", [E3, Dm], F32, kind="ExternalInput").ap()
    outw = nc.dram_tensor("outw", [Dm, Dm], F32, kind="ExternalInput").ap()
    y = nc.dram_tensor("y", [Tq, Dm], F32, kind="ExternalOutput").ap()

    # ---------------- persistent SBUF tensors ----------------
    knT, _ = _tile([P, PAIRS * Tkv], BF16, "knT")    # [hd(2 heads), s]
    qnT, _ = _tile([P, PAIRS * Tq], BF16, "qnT")     # [hd(2 heads), t]
    vbig, _ = _tile([P, TBkv * Dm], BF16, "vbig")    # natural [s, e]
    mag8, _ = _tile([P, max(TBq, 2)], F32, "mag8")   # sqrt(||x||^2*HD/D)
    ownT, _ = _tile([P, DT * Dm], BF16, "ownT")      # out_w normalized^T
    avnat, _ = _tile([P, TBq * Dm], BF16, "avnat")   # attn-out natural

    # ---------------- phase W + X + A (scoped) ----------------
    wxa = ExitStack()
    wnT, free_wnT = _tile([P, DT * E3], BF16, "wnT")
    xkvT, free_xkvT = _tile([P, DT * Tkv], BF16, "xkvT")
    wstage = wxa.enter_context(tc.tile_pool(name="wstage", bufs=2))
    xstage = wxa.enter_context(tc.tile_pool(name="xstage", bufs=3))
    sqpool = wxa.enter_context(tc.tile_pool(name="sqpool", bufs=2))
    small = wxa.enter_context(tc.tile_pool(name="small", bufs=8))
    nstage = wxa.enter_context(tc.tile_pool(name="nstage", bufs=3))
    psA = wxa.enter_context(tc.tile_pool(name="psA", bufs=2, space="PSUM"))

    def normalize_rows(dst_big, dst_stride, src_ap, we, ncols):
        """Load weight row-tile we, produce bf16 rows/(||row||+eps), transpose
        into dst_big (column-major d-tiles of width ncols)."""
        wst = wstage.tile([P, Dm], F32, name="wst", tag="wst")
        nc.sync.dma_start(wst, src_ap[we * P:(we + 1) * P, :])
        wsq = sqpool.tile([P, Dm], BF16, name="wsq", tag="sq")
        ssw = small.tile([P, 1], F32, name="ssw", tag="s1")
        nc.scalar.activation(wsq, wst, AF.Square, accum_out=ssw)
        sw = small.tile([P, 1], F32, name="sw", tag="s1")
        nc.scalar.activation(sw, ssw, AF.Sqrt)
        swe = small.tile([P, 1], F32, name="swe", tag="s1")
        nc.vector.tensor_scalar_add(swe, sw, EPS)
        rw = small.tile([P, 1], F32, name="rw", tag="s1")
        nc.vector.reciprocal(rw, swe)
        wnb = nstage.tile([P, Dm], BF16, name="wnb", tag="nst")
        nc.vector.tensor_scalar_mul(wnb, wst, rw)
        for dt in range(DT):
            nc.sync.dma_start_transpose(
                dst_big[:, dt * dst_stride + we * P: dt * dst_stride + (we + 1) * P],
                wnb[:, dt * P:(dt + 1) * P])

    for we in range(WE):
        normalize_rows(wnT, E3, qkvw, we, E3)
    for we in range(DT):
        normalize_rows(ownT, Dm, outw, we, Dm)

    # x: cast to bf16, transpose; per-token magnitude for the first TBq blocks
    for ti in range(TBkv):
        xst = xstage.tile([P, Dm], BF16, name="xst", tag="xst")
        nc.gpsimd.dma_start(xst, xkv[ti * P:(ti + 1) * P, :])  # f32 -> bf16 cast
        for dt in range(DT):
            nc.sync.dma_start_transpose(
                xkvT[:, dt * Tkv + ti * P: dt * Tkv + (ti + 1) * P],
                xst[:, dt * P:(dt + 1) * P])
        if ti < TBq:
            xsq = sqpool.tile([P, Dm], BF16, name="xsq", tag="sq")
            ssx = small.tile([P, 1], F32, name="ssx", tag="s1")
            nc.scalar.activation(xsq, xst, AF.Square, accum_out=ssx)
            nc.scalar.activation(mag8[:, ti:ti + 1], ssx, AF.Sqrt,
                                 scale=float(HDl) / float(Dm))

    # qkv projection + q/k normalization, natural layout
    def qk_normalize(ps_slice, is_k, ti):
        """ps_slice: PSUM [P, Dm] of raw q or k; returns bf16 normalized tile."""
        sqk = sqpool.tile([P, Dm], BF16, name="sqk", tag="sq")
        nc.scalar.activation(sqk, ps_slice, AF.Square)
        ssk = small.tile([P, Hn], F32, name="ssk", tag="sh")
        nc.vector.tensor_reduce(ssk, sqk.rearrange("p (h d) -> p h d", h=Hn),
                                axis=AX.X, op=ALU.add)
        sk = small.tile([P, Hn], F32, name="sk", tag="sh")
        nc.scalar.activation(sk, ssk, AF.Sqrt)
        ske = small.tile([P, Hn], F32, name="ske", tag="sh")
        if is_k:
            # fold the sqrt(HD)/HD score scale into k's factor: sqrt(HD)/(||k||+eps)
            nc.vector.tensor_scalar(ske, sk, EPS, 1.0 / math.sqrt(HDl),
                                    op0=ALU.add, op1=ALU.mult)
        else:
            nc.vector.tensor_scalar_add(ske, sk, EPS)
        rk = small.tile([P, Hn], F32, name="rk", tag="sh")
        nc.vector.reciprocal(rk, ske)
        knb = nstage.tile([P, Dm], BF16, name="knb", tag="nst")
        nc.vector.tensor_tensor(
            knb.rearrange("p (h d) -> p h d", h=Hn),
            ps_slice.rearrange("p (h d) -> p h d", h=Hn),
            rk.broadcast_to([P, Hn, HDl]),
            op=ALU.mult)
        return knb

    for ti in range(TBkv):
        # k,v for every token block
        ps = psA.tile([P, 2 * Dm], F32, name="pskv", tag="ps")
        for dt in range(DT):
            lhs = xkvT[:, dt * Tkv + ti * P: dt * Tkv + (ti + 1) * P]
            for (c0, cn) in _chunks(2 * Dm, 512):
                nc.tensor.matmul(ps[:, c0:c0 + cn], lhsT=lhs,
                                 rhs=wnT[:, dt * E3 + Dm + c0: dt * E3 + Dm + c0 + cn],
                                 start=(dt == 0), stop=(dt == DT - 1))
        knb = qk_normalize(ps[:, 0:Dm], True, ti)
        nc.scalar.activation(vbig[:, ti * Dm:(ti + 1) * Dm], ps[:, Dm:2 * Dm], AF.Copy)
        for pr in range(PAIRS):
            nc.sync.dma_start_transpose(
                knT[:, pr * Tkv + ti * P: pr * Tkv + (ti + 1) * P],
                knb[:, pr * P:(pr + 1) * P])

    for ti in range(TBq):
        # q for this core's token blocks (first TBq blocks of xkv)
        ps = psA.tile([P, Dm], F32, name="psq", tag="ps")
        for dt in range(DT):
            lhs = xkvT[:, dt * Tkv + ti * P: dt * Tkv + (ti + 1) * P]
            for (c0, cn) in _chunks(Dm, 512):
                nc.tensor.matmul(ps[:, c0:c0 + cn], lhsT=lhs,
                                 rhs=wnT[:, dt * E3 + c0: dt * E3 + c0 + cn],
                                 start=(dt == 0), stop=(dt == DT - 1))
        qnb = qk_normalize(ps[:, 0:Dm], False, ti)
        for pr in range(PAIRS):
            nc.sync.dma_start_transpose(
                qnT[:, pr * Tq + ti * P: pr * Tq + (ti + 1) * P],
                qnb[:, pr * P:(pr + 1) * P])

    wxa.close()
    free_xkvT()
    free_wnT()

    # ---------------- phase B: scores -> sigmoid -> attn @ v ----------------
    bstk = ExitStack()
    psS = bstk.enter_context(tc.tile_pool(name="psS", bufs=2, space="PSUM"))
    psAV = bstk.enter_context(tc.tile_pool(name="psAV", bufs=1, space="PSUM"))
    attnp = bstk.enter_context(tc.tile_pool(name="attnp", bufs=3))
    avtp = bstk.enter_context(tc.tile_pool(name="avtp", bufs=2))

    for pr in range(PAIRS):
        psav = psAV.tile([P, Tq], F32, name="psav", tag="psav")
        for sb in range(TBkv):
            for a in range(2):  # head within pair; rows a*64..a*64+63
                r0 = a * HDl
                pss = psS.tile([P, Tq], F32, name="pss", tag="pss")
                for (c0, cn) in _chunks(Tq, 512):
                    nc.tensor.matmul(
                        pss[:, c0:c0 + cn],
                        lhsT=knT[r0:r0 + HDl, pr * Tkv + sb * P: pr * Tkv + (sb + 1) * P],
                        rhs=qnT[r0:r0 + HDl, pr * Tq + c0: pr * Tq + c0 + cn],
                        start=True, stop=True)
                attn = attnp.tile([P, Tq], BF16, name="attn", tag="attn")
                nc.scalar.activation(attn, pss, AF.Sigmoid)
                for (c0, cn) in _chunks(Tq, 512):
                    nc.tensor.matmul(
                        psav[r0:r0 + HDl, c0:c0 + cn],
                        lhsT=vbig[:, sb * Dm + pr * P + r0: sb * Dm + pr * P + r0 + HDl],
                        rhs=attn[:, c0:c0 + cn],
                        start=(sb == 0), stop=(sb == TBkv - 1),
                        skip_group_check=True)
        avt = avtp.tile([P, Tq], BF16, name="avt", tag="avt")
        nc.vector.tensor_copy(avt, psav)
        for tb in range(TBq):
            nc.sync.dma_start_transpose(
                avnat[:, tb * Dm + pr * P: tb * Dm + (pr + 1) * P],
                avt[:, tb * P:(tb + 1) * P])
    bstk.close()

    # ---------------- phase C: normalize + magnitude + out-proj ----------------
    avnT, _ = _tile([P, DT * Tq], BF16, "avnT")
    cstk = ExitStack()
    psO = cstk.enter_context(tc.tile_pool(name="psO", bufs=2, space="PSUM"))
    sqc = cstk.enter_context(tc.tile_pool(name="sqc", bufs=2))
    smallc = cstk.enter_context(tc.tile_pool(name="smallc", bufs=8))
    avnp = cstk.enter_context(tc.tile_pool(name="avnp", bufs=2))
    ypool = cstk.enter_context(tc.tile_pool(name="ypool", bufs=2))

    for tb in range(TBq):
        src = avnat[:, tb * Dm:(tb + 1) * Dm]
        sqa = sqc.tile([P, Dm], BF16, name="sqa", tag="sqa")
        nc.vector.tensor_mul(sqa, src, src)
        ssa = smallc.tile([P, Hn], F32, name="ssa", tag="sh")
        nc.vector.tensor_reduce(ssa, sqa.rearrange("p (h d) -> p h d", h=Hn),
                                axis=AX.X, op=ALU.add)
        sa = smallc.tile([P, Hn], F32, name="sa", tag="sh")
        nc.scalar.activation(sa, ssa, AF.Sqrt)
        sae = smallc.tile([P, Hn], F32, name="sae", tag="sh")
        nc.vector.tensor_scalar_add(sae, sa, eps_av)
        ra = smallc.tile([P, Hn], F32, name="ra", tag="sh")
        nc.vector.reciprocal(ra, sae)
        g = smallc.tile([P, Hn], F32, name="g", tag="sh")
        nc.vector.tensor_scalar_mul(g, ra, mag8[:, tb:tb + 1])
        avn = avnp.tile([P, Dm], BF16, name="avn", tag="avn")
        nc.vector.tensor_tensor(
            avn.rearrange("p (h d) -> p h d", h=Hn),
            src.rearrange("p (h d) -> p h d", h=Hn),
            g.broadcast_to([P, Hn, HDl]),
            op=ALU.mult)
        for dt in range(DT):
            nc.sync.dma_start_transpose(
                avnT[:, dt * Tq + tb * P: dt * Tq + (tb + 1) * P],
                avn[:, dt * P:(dt + 1) * P])

    for tb in range(TBq):
        pso = psO.tile([P, Dm], F32, name="pso", tag="pso")
        for dt in range(DT):
            lhs = avnT[:, dt * Tq + tb * P: dt * Tq + (tb + 1) * P]
            for (c0, cn) in _chunks(Dm, 512):
                nc.tensor.matmul(pso[:, c0:c0 + cn], lhsT=lhs,
                                 rhs=ownT[:, dt * Dm + c0: dt * Dm + c0 + cn],
                                 start=(dt == 0), stop=(dt == DT - 1))
        ysb = ypool.tile([P, Dm], F32, name="ysb", tag="ysb")
        nc.vector.tensor_copy(ysb, pso)
        nc.sync.dma_start(y[tb * P:(tb + 1) * P, :], ysb)
    cstk.close()


def make_nc(Tq=T // 2, Tkv=T, Dm=D, Hn=H):
    nc = bacc.Bacc("TRN2", target_bir_lowering=False, debug=False,
                   num_devices=N_CORES)
    with ExitStack() as ctx:
        with tile.TileContext(nc) as tc:
            build_program(nc, tc, ctx, Tq, Tkv, Dm, Hn)
    nc.compile()
    return nc


_CACHED_NC = None


def _get_nc():
    global _CACHED_NC
    if _CACHED_NC is None:
        _CACHED_NC = make_nc()
    return _CACHED_NC


def _shard_inputs(x, qkv_w, out_w):
    Tq = T // 2
    x = np.asarray(x, dtype=np.float32)
    qkv_w = np.ascontiguousarray(np.asarray(qkv_w, dtype=np.float32))
    out_w = np.ascontiguousarray(np.asarray(out_w, dtype=np.float32))
    in_maps = []
    for core in range(N_CORES):
        b, half = core // 2, core % 2
        own = x[b, half * Tq:(half + 1) * Tq]
        other = x[b, (1 - half) * Tq:(2 - half) * Tq]
        xkv = np.ascontiguousarray(np.concatenate([own, other], axis=0))
        in_maps.append({"xkv": xkv, "qkvw": qkv_w, "outw": out_w})
    return in_maps


def run(x, qkv_w, out_w, trace=False, trace_cores=None):
    nc = _get_nc()
    in_maps = _shard_inputs(x, qkv_w, out_w)
    res = run_bass_kernel_spmd(nc, in_maps, list(range(N_CORES)),
                               trace=trace, trace_cores=trace_cores)
    Tq = T // 2
    y = np.empty((B, T, D), np.float32)
    for core, r in enumerate(res.results):
        b, half = core // 2, core % 2
        y[b, half * Tq:(half + 1) * Tq] = r["y"]
    return y, res


def kernel(x, qkv_w, out_w):
    y, _ = run(x, qkv_w, out_w, trace=False)
    return y


# revision 9
# speedup vs baseline: 1.4905x; 1.4905x over previous
"""Trainium2 Bass kernel for nn_Attention_4363686773373.

Sigmoid attention with magnitude-preserving (weight-normalized) projections.

Sharding: data-parallel over (batch, T-half) -> 8 shards on 8 NeuronCores.
Each core computes q for its 1024 tokens and k,v for the full 2048 tokens of
its batch (k/v recomputed on both cores of a batch; no collectives).

Per-core dataflow (all heavy matmuls in bf16 with fp32 PSUM accumulation):
  W: row-normalize qkv_w / out_w on device; bounce bf16 rows through DRAM and
     transpose with large DMA-xbar transposes -> wnT, ownT
  X: cast x to bf16 (gpsimd DRAM->DRAM cast DMA), large transposes -> xkvT;
     per-token ||x|| from f32 x -> mag
  A: qkv projection (natural [t,e] layout), q/k cosine-normalize along head_dim
     (free-dim reduce), bounce qn/kn through DRAM -> [head_dim, t] layout
  B: per head: scoresT = knT^T qnT (K=64 row-packed head pairs), sigmoid on the
     scalar engine (PSUM->SBUF bf16), attnT @ v accumulated over key blocks;
     per-pair PE-transposes bring attn-out back to natural layout
  C: normalize av per (token, head), scale by token magnitude, PE-transpose,
     out-projection.

DMA dispatch rings: sync = xbar transposes, scalar(ACT) = weight/x loads,
gpsimd(SWDGE) = DRAM scratch writes + casts + output stores.
"""

import math
from contextlib import ExitStack

import numpy as np

import concourse.bass as bass
import concourse.tile as tile
from concourse import bacc, mybir
from concourse.bass_utils import run_bass_kernel_spmd
from concourse.masks import make_identity

# Problem shapes (hardcoded per harness contract)
B, T, D, H = 4, 2048, 768, 12
HD = D // H  # 64
EPS = 1e-4
SIGMOID_GAIN = 1.8402
N_CORES = 8

F32 = mybir.dt.float32
BF16 = mybir.dt.bfloat16
AF = mybir.ActivationFunctionType
ALU = mybir.AluOpType
AX = mybir.AxisListType


def _ensure_axon_hooks():
    """This image's antenv lacks axon_hooks; reconstruct it so trace=True
    (NTFF profiling) works instead of crashing on import."""
    try:
        import antenv.axon_hooks  # noqa: F401
        return
    except ImportError:
        pass
    import sys
    import types
    try:
        import antenv
    except ImportError:
        return
    mod = types.ModuleType("antenv.axon_hooks")
    _hook = [None]
    mod.set_axon_ntff_profile_hook = lambda h: _hook.__setitem__(0, h)
    mod.get_axon_ntff_profile_hook = lambda: _hook[0]
    sys.modules["antenv.axon_hooks"] = mod
    antenv.axon_hooks = mod
    try:
        from trn_agent_boot.trn_boot import _ntff_profile_via_ctypes
        mod.set_axon_ntff_profile_hook(
            _ntff_profile_via_ctypes('/opt/axon/libaxon_pjrt.so'))
    except Exception:
        pass


_ensure_axon_hooks()


def _chunks(total, maxn=512):
    out = []
    c0 = 0
    while c0 < total:
        cn = min(maxn, total - c0)
        out.append((c0, cn))
        c0 += cn
    return out


def build_program(nc, tc, ctx, Tq, Tkv, Dm, Hn):
    """Emit the per-core program. xkv rows are pre-ordered so the first Tq
    tokens are this core's query tokens (attention is permutation-invariant
    over the key axis)."""
    keep = []  # keep tc.tile free-closures alive (GC would release the pools)

    def _tile(shape, dtype, name):
        t, free = tc.tile(shape, dtype, name=name)
        keep.append(free)
        return t, free

    tc._ant_keepalive = keep
    P = 128
    HDl = 64
    assert Dm % P == 0 and Tq % P == 0 and Tkv % P == 0
    DT = Dm // P          # d-tiles
    E3 = 3 * Dm
    PAIRS = Hn // 2       # head pairs; pair = 128 contiguous features
    assert PAIRS * P == Dm and Hn * HDl == Dm
    TBq = Tq // P
    TBkv = Tkv // P
    WE = E3 // P          # qkv_w row tiles
    # eps seen by the post-attention normalize, after folding out the
    # gain/sqrt(T) prefactor (we accumulate raw attn@v).
    eps_av = EPS * math.sqrt(Tkv) / SIGMOID_GAIN

    xkv = nc.dram_tensor("xkv", [Tkv, Dm], F32, kind="ExternalInput").ap()
    qkvw = nc.dram_tensor("qkvw", [E3, Dm], F32, kind="ExternalInput").ap()
    outw = nc.dram_tensor("outw", [Dm, Dm], F32, kind="ExternalInput").ap()
    y = nc.dram_tensor("y", [Tq, Dm], F32, kind="ExternalOutput").ap()

    # ---------------- DRAM scratch ----------------
    dstk = ExitStack()
    dpool = dstk.enter_context(tc.tile_pool(name="dram", bufs=1, space="DRAM"))
    wn_dram = dpool.tile([E3, Dm], BF16, name="wn_dram")
    own_dram = dpool.tile([Dm, Dm], BF16, name="own_dram")
    xbf_dram = dpool.tile([Tkv, Dm], BF16, name="xbf_dram")
    kn_dram = dpool.tile([Tkv, Dm], BF16, name="kn_dram")
    qn_dram = dpool.tile([Tq, Dm], BF16, name="qn_dram")

    # ---------------- persistent SBUF tensors ----------------
    knT, _ = _tile([P, PAIRS * Tkv], BF16, "knT")    # [hd(2 heads), s]
    qnT, _ = _tile([P, PAIRS * Tq], BF16, "qnT")     # [hd(2 heads), t]
    vbig, _ = _tile([P, TBkv * Dm], BF16, "vbig")    # natural [s, e]
    mag8, _ = _tile([P, max(TBq, 2)], F32, "mag8")   # sqrt(||x||^2*HD/D)
    ownT, _ = _tile([P, DT * Dm], BF16, "ownT")      # out_w normalized^T
    avnat, _ = _tile([P, TBq * Dm], BF16, "avnat")   # attn-out natural
    ident, _ = _tile([P, P], BF16, "ident")          # PE-transpose identity
    make_identity(nc, ident)

    # ---------------- phase W + X + A (scoped) ----------------
    wxa = ExitStack()
    wnT, free_wnT = _tile([P, DT * E3], BF16, "wnT")
    xkvT, free_xkvT = _tile([P, DT * Tkv], BF16, "xkvT")
    wstage = wxa.enter_context(tc.tile_pool(name="wstage", bufs=3))
    sqpool = wxa.enter_context(tc.tile_pool(name="sqpool", bufs=2))
    small = wxa.enter_context(tc.tile_pool(name="small", bufs=8))
    nstage = wxa.enter_context(tc.tile_pool(name="nstage", bufs=3))
    psA = wxa.enter_context(tc.tile_pool(name="psA", bufs=2, space="PSUM"))

    def normalize_rows(dst_dram, src_ap, we):
        """Load weight row-tile we, write bf16 rows/(||row||+eps) to DRAM."""
        wst = wstage.tile([P, Dm], F32, name="wst", tag="wst")
        nc.scalar.dma_start(wst, src_ap[we * P:(we + 1) * P, :])
        wsq = sqpool.tile([P, Dm], BF16, name="wsq", tag="sq")
        ssw = small.tile([P, 1], F32, name="ssw", tag="s1")
        nc.scalar.activation(wsq, wst, AF.Square, accum_out=ssw)
        sw = small.tile([P, 1], F32, name="sw", tag="s1")
        nc.scalar.activation(sw, ssw, AF.Sqrt)
        swe = small.tile([P, 1], F32, name="swe", tag="s1")
        nc.vector.tensor_scalar_add(swe, sw, EPS)
        rw = small.tile([P, 1], F32, name="rw", tag="s1")
        nc.vector.reciprocal(rw, swe)
        wnb = nstage.tile([P, Dm], BF16, name="wnb", tag="nst")
        nc.vector.tensor_scalar_mul(wnb, wst, rw)
        nc.gpsimd.dma_start(dst_dram[we * P:(we + 1) * P, :], wnb)

    # qkv_w -> wn_dram; transpose per (e-chunk of DT row-tiles, d-tile)
    for we in range(WE):
        normalize_rows(wn_dram, qkvw, we)
        if we % DT == DT - 1:
            ch = we // DT
            for dt in range(DT):
                nc.sync.dma_start_transpose(
                    wnT[:, dt * E3 + ch * Dm:dt * E3 + (ch + 1) * Dm],
                    wn_dram[ch * Dm:(ch + 1) * Dm, dt * P:(dt + 1) * P])
    for we in range(DT):
        normalize_rows(own_dram, outw, we)
    for dt in range(DT):
        nc.sync.dma_start_transpose(
            ownT[:, dt * Dm:(dt + 1) * Dm],
            own_dram[:, dt * P:(dt + 1) * P])

    # x: DRAM->DRAM bf16 cast, then large transposes; f32 loads for magnitude
    XCH = max(Tkv // 2, P)
    for h0 in range(0, Tkv, XCH):
        nc.gpsimd.dma_start(xbf_dram[h0:h0 + XCH, :], xkv[h0:h0 + XCH, :])
        for dt in range(DT):
            nc.sync.dma_start_transpose(
                xkvT[:, dt * Tkv + h0:dt * Tkv + h0 + XCH],
                xbf_dram[h0:h0 + XCH, dt * P:(dt + 1) * P])
    for ti in range(TBq):
        xmt = wstage.tile([P, Dm], F32, name="xmt", tag="wst")
        nc.scalar.dma_start(xmt, xkv[ti * P:(ti + 1) * P, :])
        xsq = sqpool.tile([P, Dm], BF16, name="xsq", tag="sq")
        ssx = small.tile([P, 1], F32, name="ssx", tag="s1")
        nc.scalar.activation(xsq, xmt, AF.Square, accum_out=ssx)
        nc.scalar.activation(mag8[:, ti:ti + 1], ssx, AF.Sqrt,
                             scale=float(HDl) / float(Dm))

    # qkv projection + q/k normalization, natural layout
    def qk_normalize(ps_slice, is_k):
        """ps_slice: PSUM [P, Dm] of raw q or k; returns bf16 normalized tile."""
        sqk = sqpool.tile([P, Dm], BF16, name="sqk", tag="sq")
        nc.scalar.activation(sqk, ps_slice, AF.Square)
        ssk = small.tile([P, Hn], F32, name="ssk", tag="sh")
        nc.vector.tensor_reduce(ssk, sqk.rearrange("p (h d) -> p h d", h=Hn),
                                axis=AX.X, op=ALU.add)
        sk = small.tile([P, Hn], F32, name="sk", tag="sh")
        nc.scalar.activation(sk, ssk, AF.Sqrt)
        ske = small.tile([P, Hn], F32, name="ske", tag="sh")
        if is_k:
            # fold the 1/sqrt(HD) score scale into k: sqrt(HD)/(||k||+eps)
            nc.vector.tensor_scalar(ske, sk, EPS, 1.0 / math.sqrt(HDl),
                                    op0=ALU.add, op1=ALU.mult)
        else:
            nc.vector.tensor_scalar_add(ske, sk, EPS)
        rk = small.tile([P, Hn], F32, name="rk", tag="sh")
        nc.vector.reciprocal(rk, ske)
        knb = nstage.tile([P, Dm], BF16, name="knb", tag="nst")
        nc.vector.tensor_tensor(
            knb.rearrange("p (h d) -> p h d", h=Hn),
            ps_slice.rearrange("p (h d) -> p h d", h=Hn),
            rk.broadcast_to([P, Hn, HDl]),
            op=ALU.mult)
        return knb

    KHALF = TBkv // 2
    for ti in range(TBkv):
        # k,v for every token block
        ps = psA.tile([P, 2 * Dm], F32, name="pskv", tag="ps")
        for dt in range(DT):
            lhs = xkvT[:, dt * Tkv + ti * P: dt * Tkv + (ti + 1) * P]
            for (c0, cn) in _chunks(2 * Dm, 512):
                nc.tensor.matmul(ps[:, c0:c0 + cn], lhsT=lhs,
                                 rhs=wnT[:, dt * E3 + Dm + c0: dt * E3 + Dm + c0 + cn],
                                 start=(dt == 0), stop=(dt == DT - 1))
        knb = qk_normalize(ps[:, 0:Dm], True)
        nc.scalar.activation(vbig[:, ti * Dm:(ti + 1) * Dm], ps[:, Dm:2 * Dm], AF.Copy)
        nc.gpsimd.dma_start(kn_dram[ti * P:(ti + 1) * P, :], knb)
        if ti % KHALF == KHALF - 1:
            h0 = (ti // KHALF) * KHALF * P
            hn = KHALF * P
            for pr in range(PAIRS):
                nc.sync.dma_start_transpose(
                    knT[:, pr * Tkv + h0: pr * Tkv + h0 + hn],
                    kn_dram[h0:h0 + hn, pr * P:(pr + 1) * P])

    for ti in range(TBq):
        # q for this core's token blocks (first TBq blocks of xkv)
        ps = psA.tile([P, Dm], F32, name="psq", tag="ps")
        for dt in range(DT):
            lhs = xkvT[:, dt * Tkv + ti * P: dt * Tkv + (ti + 1) * P]
            for (c0, cn) in _chunks(Dm, 512):
                nc.tensor.matmul(ps[:, c0:c0 + cn], lhsT=lhs,
                                 rhs=wnT[:, dt * E3 + c0: dt * E3 + c0 + cn],
                                 start=(dt == 0), stop=(dt == DT - 1))
        qnb = qk_normalize(ps[:, 0:Dm], False)
        nc.gpsimd.dma_start(qn_dram[ti * P:(ti + 1) * P, :], qnb)
    for pr in range(PAIRS):
        nc.sync.dma_start_transpose(
            qnT[:, pr * Tq:(pr + 1) * Tq],
            qn_dram[:, pr * P:(pr + 1) * P])

    wxa.close()
    free_xkvT()
    free_wnT()

    # ---------------- phase B: scores -> sigmoid -> attn @ v ----------------
    bstk = ExitStack()
    psS = bstk.enter_context(tc.tile_pool(name="psS", bufs=2, space="PSUM"))
    psAV = bstk.enter_context(tc.tile_pool(name="psAV", bufs=1, space="PSUM"))
    psT = bstk.enter_context(tc.tile_pool(name="psT", bufs=2, space="PSUM"))
    attnp = bstk.enter_context(tc.tile_pool(name="attnp", bufs=3))
    avtp = bstk.enter_context(tc.tile_pool(name="avtp", bufs=2))

    for pr in range(PAIRS):
        psav = psAV.tile([P, Tq], F32, name="psav", tag="psav")
        for sb in range(TBkv):
            for a in range(2):  # head within pair; rows a*64..a*64+63
                r0 = a * HDl
                pss = psS.tile([P, Tq], F32, name="pss", tag="pss")
                for (c0, cn) in _chunks(Tq, 512):
                    nc.tensor.matmul(
                        pss[:, c0:c0 + cn],
                        lhsT=knT[r0:r0 + HDl, pr * Tkv + sb * P: pr * Tkv + (sb + 1) * P],
                        rhs=qnT[r0:r0 + HDl, pr * Tq + c0: pr * Tq + c0 + cn],
                        start=True, stop=True)
                attn = attnp.tile([P, Tq], BF16, name="attn", tag="attn")
                nc.scalar.activation(attn, pss, AF.Sigmoid)
                for (c0, cn) in _chunks(Tq, 512):
                    nc.tensor.matmul(
                        psav[r0:r0 + HDl, c0:c0 + cn],
                        lhsT=vbig[:, sb * Dm + pr * P + r0: sb * Dm + pr * P + r0 + HDl],
                        rhs=attn[:, c0:c0 + cn],
                        start=(sb == 0), stop=(sb == TBkv - 1),
                        skip_group_check=True)
        avt = avtp.tile([P, Tq], BF16, name="avt", tag="avt")
        nc.vector.tensor_copy(avt, psav)
        for tb in range(TBq):
            ptt = psT.tile([P, P], BF16, name="ptt", tag="ptt")
            nc.tensor.transpose(ptt, avt[:, tb * P:(tb + 1) * P], ident)
            nc.vector.tensor_copy(avnat[:, tb * Dm + pr * P: tb * Dm + (pr + 1) * P],
                                  ptt)
    bstk.close()

    # ---------------- phase C: normalize + magnitude + out-proj ----------------
    avnT, _ = _tile([P, DT * Tq], BF16, "avnT")
    cstk = ExitStack()
    psO = cstk.enter_context(tc.tile_pool(name="psO", bufs=2, space="PSUM"))
    psT2 = cstk.enter_context(tc.tile_pool(name="psT2", bufs=2, space="PSUM"))
    sqc = cstk.enter_context(tc.tile_pool(name="sqc", bufs=2))
    smallc = cstk.enter_context(tc.tile_pool(name="smallc", bufs=8))
    avnp = cstk.enter_context(tc.tile_pool(name="avnp", bufs=2))
    ypool = cstk.enter_context(tc.tile_pool(name="ypool", bufs=2))

    for tb in range(TBq):
        src = avnat[:, tb * Dm:(tb + 1) * Dm]
        sqa = sqc.tile([P, Dm], BF16, name="sqa", tag="sqa")
        nc.vector.tensor_mul(sqa, src, src)
        ssa = smallc.tile([P, Hn], F32, name="ssa", tag="sh")
        nc.vector.tensor_reduce(ssa, sqa.rearrange("p (h d) -> p h d", h=Hn),
                                axis=AX.X, op=ALU.add)
        sa = smallc.tile([P, Hn], F32, name="sa", tag="sh")
        nc.scalar.activation(sa, ssa, AF.Sqrt)
        sae = smallc.tile([P, Hn], F32, name="sae", tag="sh")
        nc.vector.tensor_scalar_add(sae, sa, eps_av)
        ra = smallc.tile([P, Hn], F32, name="ra", tag="sh")
        nc.vector.reciprocal(ra, sae)
        g = smallc.tile([P, Hn], F32, name="g", tag="sh")
        nc.vector.tensor_scalar_mul(g, ra, mag8[:, tb:tb + 1])
        avn = avnp.tile([P, Dm], BF16, name="avn", tag="avn")
        nc.vector.tensor_tensor(
            avn.rearrange("p (h d) -> p h d", h=Hn),
            src.rearrange("p (h d) -> p h d", h=Hn),
            g.broadcast_to([P, Hn, HDl]),
            op=ALU.mult)
        for dt in range(DT):
            ptt = psT2.tile([P, P], BF16, name="ptt2", tag="ptt2")
            nc.tensor.transpose(ptt, avn[:, dt * P:(dt + 1) * P], ident)
            nc.vector.tensor_copy(avnT[:, dt * Tq + tb * P: dt * Tq + (tb + 1) * P],
                                  ptt)
        # out-projection for this token block
        pso = psO.tile([P, Dm], F32, name="pso", tag="pso")
        for dt in range(DT):
            lhs = avnT[:, dt * Tq + tb * P: dt * Tq + (tb + 1) * P]
            for (c0, cn) in _chunks(Dm, 512):
                nc.tensor.matmul(pso[:, c0:c0 + cn], lhsT=lhs,
                                 rhs=ownT[:, dt * Dm + c0: dt * Dm + c0 + cn],
                                 start=(dt == 0), stop=(dt == DT - 1))
        ysb = ypool.tile([P, Dm], F32, name="ysb", tag="ysb")
        nc.vector.tensor_copy(ysb, pso)
        nc.gpsimd.dma_start(y[tb * P:(tb + 1) * P, :], ysb)
    cstk.close()
    dstk.close()


def make_nc(Tq=T // 2, Tkv=T, Dm=D, Hn=H):
    nc = bacc.Bacc("TRN2", target_bir_lowering=False, debug=False,
                   num_devices=N_CORES)
    with ExitStack() as ctx:
        with tile.TileContext(nc) as tc:
            build_program(nc, tc, ctx, Tq, Tkv, Dm, Hn)
    nc.compile()
    return nc


_CACHED_NC = None


def _get_nc():
    global _CACHED_NC
    if _CACHED_NC is None:
        _CACHED_NC = make_nc()
    return _CACHED_NC


def _shard_inputs(x, qkv_w, out_w):
    Tq = T // 2
    x = np.asarray(x, dtype=np.float32)
    qkv_w = np.ascontiguousarray(np.asarray(qkv_w, dtype=np.float32))
    out_w = np.ascontiguousarray(np.asarray(out_w, dtype=np.float32))
    in_maps = []
    for core in range(N_CORES):
        b, half = core // 2, core % 2
        own = x[b, half * Tq:(half + 1) * Tq]
        other = x[b, (1 - half) * Tq:(2 - half) * Tq]
        xkv = np.ascontiguousarray(np.concatenate([own, other], axis=0))
        in_maps.append({"xkv": xkv, "qkvw": qkv_w, "outw": out_w})
    return in_maps


def run(x, qkv_w, out_w, trace=False, trace_cores=None):
    nc = _get_nc()
    in_maps = _shard_inputs(x, qkv_w, out_w)
    res = run_bass_kernel_spmd(nc, in_maps, list(range(N_CORES)),
                               trace=trace, trace_cores=trace_cores)
    Tq = T // 2
    y = np.empty((B, T, D), np.float32)
    for core, r in enumerate(res.results):
        b, half = core // 2, core % 2
        y[b, half * Tq:(half + 1) * Tq] = r["y"]
    return y, res


def kernel(x, qkv_w, out_w):
    y, _ = run(x, qkv_w, out_w, trace=False)
    return y
